# revision 8
# baseline (speedup 1.0000x reference)
"""DetConB loss (nn_DetConBLoss) on 8 TRN2 NeuronCores via Bass/Tile.

Strategy (data-parallel over batch, targets replicated):
  - Host: l2-normalize preds/targets in f32, flatten to (4096, 256),
    transpose to (d, rows), cast fp8e4m3. Core c owns pred rows
    [c*512, (c+1)*512). Each core receives the full targets with columns
    rolled by c*512 so its own-image diagonal band sits at a fixed,
    compile-time-constant column range (the program is SPMD-identical).
  - Device (per core): 50 units, each a (128 pred x W target) slab:
    fp8 DoubleRow matmuls (K=256 in one pass, f32 PSUM) + one of two
    row-sum consumers, statically balanced to the engines' measured
    rates:
      * 28 ACT units (W=1536, ~1.76us each): exp via ScalarE ACTIVATE
        with the free in-op accumulator (accum_out) - one fused pass.
      * 22 DVE units (W=1024, ~2.26us each): Schraudolph fast-exp on
        DVE (int-converting multiply-add to an i32 whose bits are the
        f32 exp) + bitcast tensor_reduce. Placed on target slabs that
        exclude both own-image diagonal bands, so the -inf correction
        on the host subtracts exact exps.
    PSUM: ACT ping-pongs over its own 2x[128,1536] slots (refill hides
    under the alternate slot's ACTIVATE); DVE needs only ONE [128,1024]
    slot because its refill (2 matmuls, ~0.5us) hides under the
    tensor_reduce of the previous unit, which reads SBUF, not PSUM.
    Total 2*3 + 2 = 8 banks. No slot is ever handed between engines.
  - Host: the 16x16 own-image diagonal dot blocks (recomputed from the
    same fp8 inputs, ~0.4% of total FLOPs), masks from the roi indices,
    positive-pair sums, the -inf masking correction (subtract the exp of
    masked entries from the denominators), log, and the final mean.
"""
import numpy as np
import ml_dtypes

import concourse.bacc as bacc
import concourse.mybir as mybir
import concourse.tile as tile
from concourse.bass_utils import run_bass_kernel_spmd

TEMP = 0.1
EPS = 1e-11
SCALE = float(np.float32(1.0 / (TEMP + EPS)))
NCORES = 8
B, N, D = 256, 16, 256
R = B * N          # 4096 flat rows
RPC = R // NCORES  # 512 rows per core
BF16 = mybir.dt.bfloat16
FP8 = mybir.dt.float8e4
NPFP8 = ml_dtypes.float8_e4m3
F32 = mybir.dt.float32
I32 = mybir.dt.int32
# Schraudolph fast-exp: exp(s*x) ~= bitcast_f32(int32(x*SA + SB))
SA = float(np.float32((2**23 / np.log(2.0)) * (1.0 / (0.1 + 1e-11))))
SB = float(np.float32(127 * 2**23 - 486411))

# Per (pt, tsel) side (4096 target cols): either [A:0-1536, A:1536-3072,
# D:3072-4096] or 4 D slabs of 1024.  The all-D sides must avoid the
# own-image diagonal (aa: t1 cols [0,512) for px=0, bb: t2 for px=1).
ALL_D_SIDES = {(3, 1), (7, 0)}
# unit = (pt, tsel, c0, w, kind)
UNITS = []
for _pt in range(8):
    for _ts in range(2):
        if (_pt, _ts) in ALL_D_SIDES:
            UNITS += [(_pt, _ts, c, 1024, "D") for c in range(0, 4096, 1024)]
        else:
            UNITS += [(_pt, _ts, 0, 1536, "A"), (_pt, _ts, 1536, 1536, "A"),
                      (_pt, _ts, 3072, 1024, "D")]
A_COST, D_COST = 1755, 2280


def unit_sequence():
    """Merge A and D unit queues by cumulative engine time so both engines
    are fed from the start and finish together."""
    a = [u for u in UNITS if u[4] == "A"]
    d = [u for u in UNITS if u[4] == "D"]
    seq, ta, td = [], 0, 0
    while a or d:
        if a and (not d or ta + A_COST <= td + D_COST):
            seq.append(a.pop(0))
            ta += A_COST
        else:
            seq.append(d.pop(0))
            td += D_COST
    return seq


def build_nc():
    """Build + schedule + compile the SPMD per-core Bass program."""
    nc = bacc.Bacc("TRN2", target_bir_lowering=False, debug=False,
                   num_devices=NCORES)

    # pred layout: [128 K-part, pt*256 + k*128 + col] so a pred tile is one
    # 256B-contiguous-per-partition DMA.
    p_dram = nc.dram_tensor("pt", [128, 2048], FP8, kind="ExternalInput")
    t_dram = [nc.dram_tensor(f"t{i + 1}t", [D, R], FP8, kind="ExternalInput")
              for i in range(2)]
    sacc = nc.dram_tensor("sacc", [128, 64], F32, kind="ExternalOutput")

    seq = unit_sequence()
    ucol = {u[:3]: i for i, u in enumerate(seq)}

    with tile.TileContext(nc) as tc:
        with (
            tc.tile_pool(name="const", bufs=1) as const_pool,
            tc.tile_pool(name="psum", bufs=1, space="PSUM") as psum_pool,
            tc.tile_pool(name="sch", bufs=2) as sch_pool,
        ):
            # Persistent SBUF: targets as [K=128 partitions, kchunk*R + col].
            t_sb = [const_pool.tile([128, 2 * R], FP8, name=f"t_sb{i}", tag=f"t{i}")
                    for i in range(2)]
            p_sb = const_pool.tile([128, 2048], FP8, name="p_sb", tag="p")
            t3 = [t_sb[i].rearrange("p (k c) -> p k c", k=2) for i in range(2)]
            p4 = p_sb.rearrange("p (t k c) -> p t k c", t=8, k=2)

            strip = const_pool.tile([128, 64], F32, name="strip", tag="strip")
            zbias = const_pool.tile([128, 1], F32, name="zbias", tag="zbias")
            scr = const_pool.tile([128, 1536], BF16, name="scr", tag="scr")
            nc.vector.memset(strip, 0.0)
            # Explicit zero-bias AP: a float bias would be lowered through the
            # const-AP machinery, whose TENSOR_LOAD sits in the preamble.
            nc.vector.memset(zbias, 0.0)

            # PSUM: ACT pair 2x[128,1536] (6 banks) + one DVE slot (2 banks).
            psA = [psum_pool.tile([128, 1536], F32, name=f"psA{i}", tag=f"psA{i}")
                   for i in range(2)]
            psD = psum_pool.tile([128, 1024], F32, name="psD", tag="psD")

            # Input DMAs, need-ordered.  The handful of chunks gating the
            # first two units go on the ACT HWDGE queue (a few ~80ns issue
            # slots before ACT's first EXP); the bulk streams on the SP
            # queue in 2048-col chunks (2KB descriptors).
            def load_p(pt, eng):
                eng.dma_start(out=p_sb[:, pt * 256:(pt + 1) * 256],
                              in_=p_dram[:, pt * 256:(pt + 1) * 256])

            def load_t(tsel, k, c0, c1, eng):
                eng.dma_start(
                    out=t_sb[tsel][:, k * R + c0: k * R + c1],
                    in_=t_dram[tsel][k * 128:(k + 1) * 128, c0:c1])

            # Need-ordered 1024-col subchunk list (ts, k, c) + pred tiles.
            need_t, need_p = [], []
            for pt, ts, c0, w, _ in seq:
                if pt not in need_p:
                    need_p.append(pt)
                lo, hi = (c0 // 1024) * 1024, min(-(-(c0 + w) // 1024) * 1024, R)
                for c in range(lo, hi, 1024):
                    for k in range(2):
                        if (ts, k, c) not in need_t:
                            need_t.append((ts, k, c))
            loaded = set()
            # chunks gating the first two units go on the ACT queue
            load_p(need_p[0], nc.scalar)
            for ts, k, c in need_t[:6]:
                loaded.add((ts, k, c))
                load_t(ts, k, c, c + 1024, nc.scalar)
            # the rest on the SP queue, merged into up-to-2048-col runs
            for pt in need_p[1:]:
                load_p(pt, nc.sync)
            i = 0
            while i < len(need_t):
                ts, k, c = need_t[i]
                i += 1
                if (ts, k, c) in loaded:
                    continue
                loaded.add((ts, k, c))
                c1 = c + 1024
                if (i < len(need_t) and need_t[i] == (ts, k, c1)
                        and (ts, k, c1) not in loaded):
                    loaded.add((ts, k, c1))
                    c1 += 1024
                    i += 1
                load_t(ts, k, c, c1, nc.sync)

            # Warm the exp table set during the input-DMA window so the first
            # real ACTIVATE does not pay the ~2.7us ACT_TABLE_LOAD.
            nc.scalar.activation(strip[:, 0:2], strip[:, 0:2],
                                 mybir.ActivationFunctionType.Exp, bias=zbias)
            nc.vector.memset(strip[:, 0:2], 0.0)

            na = 0
            for i, (pt, ts, c0, w, kind) in enumerate(seq):
                ps = psA[na % 2] if kind == "A" else psD
                lhs = p4[:, pt, :, :]
                # fp8 DoubleRow: both 128-deep K chunks contract in a single
                # pass (lhsT/rhs carry the k pair on a middle AP dim).
                for j in range(0, w, 512):
                    nc.tensor.matmul(
                        ps[:, j:j + 512],
                        lhs, t3[ts][:, :, c0 + j:c0 + j + 512],
                        start=True, stop=True,
                        perf_mode=mybir.MatmulPerfMode.DoubleRow)
                if kind == "A":
                    nc.scalar.activation(
                        scr, ps, mybir.ActivationFunctionType.Exp,
                        bias=zbias, scale=SCALE,
                        accum_out=strip[:, i:i + 1])
                    na += 1
                else:
                    sch = sch_pool.tile([128, 1024], I32, name="sch", tag="sch")
                    nc.vector.tensor_scalar(
                        sch, ps, SA, SB,
                        op0=mybir.AluOpType.mult, op1=mybir.AluOpType.add)
                    nc.vector.tensor_reduce(
                        strip[:, i:i + 1], sch.bitcast(F32),
                        axis=mybir.AxisListType.X, op=mybir.AluOpType.add)
            # Final strip DMA on the sync HWDGE queue: drains in ~0.1us at
            # kernel exit (the gpsimd SWDGE path would take ~2.4us).
            nc.sync.dma_start(out=sacc.ap(), in_=strip)

    nc.compile()
    return nc, ucol


_NC = None


def _get_nc():
    global _NC
    if _NC is None:
        _NC = build_nc()
    return _NC


def _l2norm(x):
    return x / np.linalg.norm(x, axis=-1, keepdims=True)


def host_prep(pred1, pred2, target1, target2):
    p1t = _l2norm(np.asarray(pred1, np.float32)).reshape(R, D).T.astype(NPFP8)
    p2t = _l2norm(np.asarray(pred2, np.float32)).reshape(R, D).T.astype(NPFP8)
    t1t = _l2norm(np.asarray(target1, np.float32)).reshape(R, D).T.astype(NPFP8)
    t2t = _l2norm(np.asarray(target2, np.float32)).reshape(R, D).T.astype(NPFP8)
    # Raw own-image diagonal dot blocks (b, n, m), fp8-quantized operands in
    # f32 - the same products the device computes, ~0.4% of total FLOPs.
    pf = [p1t.T.astype(np.float32).reshape(B, N, D),
          p2t.T.astype(np.float32).reshape(B, N, D)]
    tf = [t1t.T.astype(np.float32).reshape(B, N, D),
          t2t.T.astype(np.float32).reshape(B, N, D)]
    diag = [[np.einsum('bnd,bmd->bnm', pf[px], tf[ts]).astype(np.float32)
             for ts in range(2)] for px in range(2)]
    in_maps = []
    for c in range(NCORES):
        r0 = c * RPC
        # pred layout [128 K-part, pt*256 + k*128 + col], pt = px*4 + mt
        pcat = np.concatenate([p1t[:, r0:r0 + RPC], p2t[:, r0:r0 + RPC]],
                              axis=1)                      # [256, 1024]
        pk = pcat.reshape(2, 128, 8, 128)                  # [k, part, pt, col]
        pk = pk.transpose(1, 2, 0, 3).reshape(128, 2048)   # [part, pt*k*col]
        in_maps.append({
            "pt": np.ascontiguousarray(pk),
            "t1t": np.ascontiguousarray(
                np.concatenate([t1t[:, r0:], t1t[:, :r0]], axis=1)),
            "t2t": np.ascontiguousarray(
                np.concatenate([t2t[:, r0:], t2t[:, :r0]], axis=1)),
        })
    return in_maps, diag


def host_post(results, ucol, diag, pind1, pind2, tind1, tind2):
    # S[px, pred]: denominator sums of exp over all 8192 targets.
    S = np.zeros((2, R), np.float64)
    cols = {pt: [c for (p2_, t2_, c0), c in ucol.items() if p2_ == pt]
            for pt in range(8)}
    for c, res in enumerate(results):
        r0 = c * RPC
        sa = np.asarray(res["sacc"]).astype(np.float64)   # [128, 64]
        for pt in range(8):
            px, mt = pt // 4, pt % 4
            rows = r0 + mt * 128
            S[px, rows:rows + 128] += sa[:, cols[pt]].sum(axis=1)

    sc = np.float32(SCALE)
    D_aa = sc * diag[0][0]
    D_ab = sc * diag[0][1]
    D_ba = sc * diag[1][0]
    D_bb = sc * diag[1][1]

    f32 = np.float32
    pind1, pind2 = np.asarray(pind1), np.asarray(pind2)
    tind1, tind2 = np.asarray(tind1), np.asarray(tind2)
    same_aa = (pind1[:, :, None] == tind1[:, None, :]).astype(f32)
    same_ab = (pind1[:, :, None] == tind2[:, None, :]).astype(f32)
    same_ba = (pind2[:, :, None] == tind1[:, None, :]).astype(f32)
    same_bb = (pind2[:, :, None] == tind2[:, None, :]).astype(f32)

    S0 = S[0].reshape(B, N)
    S1 = S[1].reshape(B, N)
    # -inf masking correction: both diagonal bands live on ACT units, so
    # the device added exact exps - subtract exact exps.
    corr0 = (same_aa * np.exp(D_aa.astype(np.float64))).sum(-1)
    corr1 = (same_bb * np.exp(D_bb.astype(np.float64))).sum(-1)
    lse0 = np.log(S0 - corr0)
    lse1 = np.log(S1 - corr1)

    num_pos0 = same_ab.sum(-1)
    num_pos1 = same_ba.sum(-1)
    pos_sum0 = (same_ab * D_ab).sum(-1)
    pos_sum1 = (same_ba * D_ba).sum(-1)

    area0 = (pind1[:, :, None] == pind1[:, None, :]).astype(f32).sum(-1)
    area1 = (pind2[:, :, None] == pind2[:, None, :]).astype(f32).sum(-1)
    w0 = (num_pos0 > 0.001).astype(f32) / area0
    w1 = (num_pos1 > 0.001).astype(f32) / area1

    ce0 = -w0 * (pos_sum0 - num_pos0 * lse0) / np.maximum(num_pos0, 1.0)
    ce1 = -w1 * (pos_sum1 - num_pos1 * lse1) / np.maximum(num_pos1, 1.0)
    return np.float32(ce0.mean() + ce1.mean())


def run_hw(inputs, trace=False):
    nc, ucol = _get_nc()
    in_maps, diag = host_prep(inputs["pred1"], inputs["pred2"],
                              inputs["target1"], inputs["target2"])
    last_err = None
    for attempt in range(3):
        try:
            res = run_bass_kernel_spmd(nc, in_maps,
                                       core_ids=list(range(NCORES)),
                                       trace=trace)
            break
        except Exception as e:  # transient NRT device errors recover on retry
            last_err = e
            import time
            time.sleep(20 * (attempt + 1))
    else:
        raise last_err
    loss = host_post(res.results, ucol, diag, inputs["pind1"],
                     inputs["pind2"], inputs["tind1"], inputs["tind2"])
    return loss, res


def kernel(**inputs):
    loss, _ = run_hw(inputs, trace=False)
    return loss
